# revision 18
# baseline (speedup 1.0000x reference)
"""Trainium2 Bass kernel for nn_ClockAwareGNN (segment_reduce).

Model (reference, fp32):
    gp   = segment_mean(x, batch) @ W_base + b_base            # [B, 1]
    h    = relu(clock @ W1 + b1) @ W2 + b2                     # [N, 16]
    cp   = segment_mean(h, batch)                              # [B, 16]
    out  = relu([gp | cp] @ W3 + b3) @ W4 + b4                 # [B, 1]

Everything after the segment reductions is affine in per-graph quantities, so
the heavy per-node work collapses to fused segment sums:
    Sx[g]  = sum of x rows in graph g          (128 cols)
    Sr[g]  = sum of r rows in graph g          (R cols)
where r is the raw clock (R=1; exact when b1 == 0 and clock >= 0 since
relu(c*W1) == c*relu(W1) elementwise for c >= 0) or relu(clock @ W1 + b1)
(R=hid fallback). Counts are host-side bincounts (shipped as 1/cnt).

Device strategy (per core, 8-way data-parallel by graph):
  - the whole payload [x | r] ships as ONE fp8(e4m3) stream: 129 B/node.
    End-to-end quantization error is ~2.5e-3 of output scale (measured on
    the real input distribution) vs the 2e-2 gate.
  - canonical slot layout: every 32-graph window is laid out as 32
    FIXED-size node slots (slot j sized to the j-th largest graph over all
    windows; each window's graphs are placed into slots sorted by size,
    host unpermutes the gathered output).  The tile -> segment map is then
    IDENTICAL on every core, so the one-hot "assign" weights become a tiny
    static fp8 table indexed by compile-time AP offsets — no per-node
    assignment work on the device at all.
  - PE accumulates assign.T @ payload into PSUM [128 graphs, 129] fp32.
    Tiles are interleaved across the 4 windows (tile i -> window i%4) so
    consecutive matmuls land in different PSUM col-groups and overlap in
    the array (tile_position col packing).
  - tiny vector-engine epilogue computes the folded per-graph MLP.
"""

import math
import sys
import types

import numpy as np
import ml_dtypes

import concourse.bass as bass
import concourse.bacc as bacc
import concourse.tile as tile
from concourse import mybir
from concourse.bass_utils import run_bass_kernel_spmd


def _ensure_axon_hooks():
    """bass_utils' trace path does `from antenv.axon_hooks import ...`;
    some agent images lack that submodule. Install it (with the real NTFF
    hook when available) so trace=True degrades gracefully instead of
    raising ModuleNotFoundError."""
    try:
        import antenv  # noqa: F401
        import antenv.axon_hooks  # noqa: F401
        return
    except ImportError:
        pass
    try:
        import antenv
    except ImportError:
        return
    mod = types.ModuleType("antenv.axon_hooks")
    state = {"hook": None}
    mod.set_axon_ntff_profile_hook = lambda h: state.__setitem__("hook", h)
    mod.get_axon_ntff_profile_hook = lambda: state["hook"]
    sys.modules["antenv.axon_hooks"] = mod
    antenv.axon_hooks = mod
    try:
        from trn_agent_boot.trn_boot import _ntff_profile_via_ctypes
        mod.set_axon_ntff_profile_hook(
            _ntff_profile_via_ctypes("/opt/axon/libaxon_pjrt.so"))
    except Exception:
        pass
    # the trace path also uploads the NEFF dir to a bucket; in zero-egress
    # containers that raises — fall back to the local path.
    try:
        import concourse.bass_utils as _bu
        _orig_upload = _bu.upload_artifacts

        def _safe_upload(tmpdir):
            try:
                return _orig_upload(tmpdir)
            except Exception:
                return str(tmpdir)

        _bu.upload_artifacts = _safe_upload
    except Exception:
        pass


_ensure_axon_hooks()

BF16 = ml_dtypes.bfloat16
F8 = ml_dtypes.float8_e4m3

N_CORES = 8
N_GRAPHS = 1024
D = 128                 # feature dim of x
GPC = N_GRAPHS // N_CORES   # graphs per core = 128
W = 32                  # slots per window (PSUM partition alignment unit)
WPC = GPC // W          # windows per core = 4
N_WIN = N_CORES * WPC   # total windows = 32


def _build_program(S, ST_, C, R, NBLK, blk):
    """Build the SPMD Bass/Tile program. Shapes and the tile -> assign-block
    map are static; per-core data differences live entirely in the inputs.

    S:    number of super-tiles (each ST_ node-tiles of 128 nodes)
    C:    fp8 payload column count = 128 + R
    NBLK: number of distinct assign-weight blocks in the table
    blk:  per-in-window-tile block index (len T_w); tile i uses blk[i//WPC]
    """
    fp32 = mybir.dt.float32
    bf16 = mybir.dt.bfloat16
    f8 = mybir.dt.float8e4
    n_tiles = S * ST_

    nc = bacc.Bacc("TRN2", target_bir_lowering=False, debug=False,
                   num_devices=N_CORES)

    xcc = nc.dram_tensor("xcc", [S, 128, ST_ * C], f8, kind="ExternalInput").ap()
    tab_d = nc.dram_tensor("tab", [128, NBLK * W], bf16, kind="ExternalInput").ap()
    rec_d = nc.dram_tensor("rec_d", [128, 1], fp32, kind="ExternalInput").ap()
    wbase_b = nc.dram_tensor("wbase_b", [128, D], fp32, kind="ExternalInput").ap()
    v1_b = nc.dram_tensor("v1_b", [128, 32], fp32, kind="ExternalInput").ap()
    m2_b = nc.dram_tensor("m2_b", [128, R * 32], fp32, kind="ExternalInput").ap()
    v0_b = nc.dram_tensor("v0_b", [128, 32], fp32, kind="ExternalInput").ap()
    w4_b = nc.dram_tensor("w4_b", [128, 32], fp32, kind="ExternalInput").ap()
    bb_t = nc.dram_tensor("bb_t", [128, 1], fp32, kind="ExternalInput").ap()
    b4_t = nc.dram_tensor("b4_t", [128, 1], fp32, kind="ExternalInput").ap()
    out_d = nc.dram_tensor("out", [128, 1], fp32, kind="ExternalOutput").ap()

    with tile.TileContext(nc) as tc:
        with (
            tc.tile_pool(name="consts", bufs=1) as cpool,
            tc.tile_pool(name="xin", bufs=8) as xpool,
            tc.tile_pool(name="epi", bufs=1) as epool,
            tc.tile_pool(name="ps", bufs=1, space="PSUM") as ppool,
        ):
            # ---- assign table first: every matmul needs it.  It rides the
            # sync ring (measured the slower of the two), which also gets one
            # fewer payload super-tile to keep the rings byte-balanced. ----
            tab_t = cpool.tile([128, NBLK * W], bf16, tag="tab")
            nc.sync.dma_start(tab_t[:], tab_d)

            psum = ppool.tile([128, C], fp32, tag="acc")

            # init matmul: zero weights x zero rhs, start=True claims the
            # whole bank's has_written bits so all later matmuls (start=False)
            # overwrite-on-first-touch / accumulate-after, independent of
            # window interleaving.
            zw = cpool.tile([128, 128], bf16, tag="zw")
            nc.vector.memset(zw[:], 0.0)
            zr = cpool.tile([128, C], bf16, tag="zr")
            nc.vector.memset(zr[:], 0.0)
            nc.tensor.matmul(psum[:, :], zw[:], zr[:], start=True, stop=False)

            # ---- main loop ----
            for s in range(S):
                xt = xpool.tile([128, ST_ * C], f8, tag="xt")
                # alternate the two HWDGE rings; scalar (faster) leads and
                # carries the final super-tile
                eng = nc.scalar if (s % 2 == 0) else nc.sync
                eng.dma_start(xt[:], xcc[s])
                for t in range(ST_):
                    i = s * ST_ + t
                    w = i % WPC       # window interleave: spread col-groups
                    b = blk[i // WPC]  # static assign-weight block
                    last = i == n_tiles - 1
                    nc.tensor.matmul(
                        psum[w * W : (w + 1) * W, :],
                        tab_t[:, b * W : (b + 1) * W],
                        xt[:, t * C : (t + 1) * C],
                        start=False,
                        stop=last,
                        tile_position=(0, w * W),
                    )

            # ---- epilogue constants (issued after the payload DMAs so the
            # scalar HWDGE ring isn't blocked at kernel start; they complete
            # long before the last matmul retires) ----
            rec_t = cpool.tile([128, 1], fp32, tag="rec")
            nc.sync.dma_start(rec_t[:], rec_d)
            wb_t = cpool.tile([128, D], fp32, tag="wb")
            nc.sync.dma_start(wb_t[:], wbase_b)
            v1_t = cpool.tile([128, 32], fp32, tag="v1")
            nc.sync.dma_start(v1_t[:], v1_b)
            m2_t = cpool.tile([128, R * 32], fp32, tag="m2")
            nc.sync.dma_start(m2_t[:], m2_b)
            v0_t = cpool.tile([128, 32], fp32, tag="v0")
            nc.sync.dma_start(v0_t[:], v0_b)
            w4_t = cpool.tile([128, 32], fp32, tag="w4")
            nc.sync.dma_start(w4_t[:], w4_b)
            bbt = cpool.tile([128, 1], fp32, tag="bb")
            nc.sync.dma_start(bbt[:], bb_t)
            b4t = cpool.tile([128, 1], fp32, tag="b4")
            nc.sync.dma_start(b4t[:], b4_t)

            # ---- epilogue (per-graph folded MLP) ----
            sb = epool.tile([128, C], fp32, tag="sb")
            nc.vector.tensor_copy(sb[:], psum[:])

            mx = epool.tile([128, D], fp32, tag="mx")
            nc.vector.tensor_scalar_mul(mx[:], sb[:, 0:D], rec_t[:])
            mr = epool.tile([128, R], fp32, tag="mr")
            nc.vector.tensor_scalar_mul(mr[:], sb[:, D : D + R], rec_t[:])

            # gp = rowsum(mean_x * W_base) + b_base
            t1 = epool.tile([128, D], fp32, tag="t1")
            nc.vector.tensor_mul(t1[:], mx[:], wb_t[:])
            gp = epool.tile([128, 1], fp32, tag="gp")
            nc.vector.tensor_reduce(gp[:], t1[:], axis=mybir.AxisListType.X,
                                    op=mybir.AluOpType.add)
            nc.vector.tensor_add(gp[:], gp[:], bbt[:])

            # pre = gp*v1 + sum_j mr[:,j]*M2[j] + v0
            pre = epool.tile([128, 32], fp32, tag="pre")
            nc.vector.tensor_scalar_mul(pre[:], v1_t[:], gp[:])
            tmp = epool.tile([128, 32], fp32, tag="tmp")
            for j in range(R):
                nc.vector.tensor_scalar(
                    tmp[:], m2_t[:, j * 32 : (j + 1) * 32], mr[:, j : j + 1], None,
                    op0=mybir.AluOpType.mult,
                )
                nc.vector.tensor_add(pre[:], pre[:], tmp[:])
            nc.vector.tensor_add(pre[:], pre[:], v0_t[:])

            act = epool.tile([128, 32], fp32, tag="act")
            nc.scalar.activation(act[:], pre[:], mybir.ActivationFunctionType.Relu)

            # out = rowsum(act * W4) + b4
            nc.vector.tensor_mul(act[:], act[:], w4_t[:])
            oo = epool.tile([128, 1], fp32, tag="oo")
            nc.vector.tensor_reduce(oo[:], act[:], axis=mybir.AxisListType.X,
                                    op=mybir.AluOpType.add)
            nc.vector.tensor_add(oo[:], oo[:], b4t[:])

            nc.sync.dma_start(out_d, oo[:])

    nc.compile()
    return nc


def kernel(x, clock_period, batch, W_base, b_base, W1, b1, W2, b2, W3, b3, W4, b4,
           _profile=None):
    x = np.asarray(x, np.float32)
    clock = np.asarray(clock_period, np.float32).reshape(-1)
    batch = np.asarray(batch, np.int32)
    W_base = np.asarray(W_base, np.float32)
    W1 = np.asarray(W1, np.float32); b1 = np.asarray(b1, np.float32)
    W2 = np.asarray(W2, np.float32); b2 = np.asarray(b2, np.float32)
    W3 = np.asarray(W3, np.float32); b3 = np.asarray(b3, np.float32)
    W4 = np.asarray(W4, np.float32); b4 = np.asarray(b4, np.float32)
    hid = W1.shape[1]

    # r-path: exact algebraic fold when relu(c*W1 + b1) == c * relu(W1)
    fold = bool(np.all(b1 == 0.0)) and bool(clock.min() >= 0.0)
    if fold:
        R = 1
        r32 = clock[:, None]                                   # [N, 1]
        q = np.maximum(W1, 0.0) @ W2                           # [1, hid]
        M2 = q @ W3[1:, :]                                     # [1, 32]
        v0 = b2 @ W3[1:, :] + b3                               # [32]
    else:
        R = hid
        r32 = np.maximum(clock[:, None] @ W1 + b1, 0.0)        # [N, hid]
        M2 = W2 @ W3[1:, :]                                    # [hid, 32]
        v0 = b2 @ W3[1:, :] + b3

    C = D + R               # fp8 payload: [x | r]

    # ---- canonical slot layout ----
    # graph node spans + per-window size-sorted slot assignment
    gcut = np.searchsorted(batch, np.arange(N_GRAPHS + 1))
    sizes = np.diff(gcut)                                      # [1024]
    sz_w = sizes.reshape(N_WIN, W)
    order = np.argsort(-sz_w, axis=1, kind="stable")           # rank -> local graph
    sorted_sz = -np.sort(-sz_w, axis=1)                        # [N_WIN, W] desc
    L = sorted_sz.max(axis=0).astype(np.int64)                 # slot sizes [W]
    B = np.zeros(W + 1, np.int64)
    B[1:] = np.cumsum(L)                                       # slot starts
    # super-tile size: ST must be a multiple of 32 so the per-partition DMA
    # line (ST*C bytes, C odd) stays 16-byte aligned for both ring halves —
    # misaligned lines fragment the SDMA descriptors (6x, measured).
    T_w = int(math.ceil(B[W] / 128.0))
    ST = 64
    while (WPC * T_w) % ST:
        T_w += 1
    n_tiles = WPC * T_w
    S = n_tiles // ST

    # static per-tile assign pattern: node offset o -> slot searchsorted(B,o)
    offs = np.arange(T_w * 128, dtype=np.int64)
    col_of = np.clip(np.searchsorted(B, offs, side="right") - 1, 0, W - 1)
    col_of = col_of.reshape(T_w, 128).astype(np.int32)
    pats = {}
    blk = np.empty(T_w, np.int32)
    for t in range(T_w):
        key = col_of[t].tobytes()
        if key not in pats:
            pats[key] = len(pats)
        blk[t] = pats[key]
    NBLK = len(pats)
    tab = np.zeros((128, NBLK * W), BF16)
    for key, b in pats.items():
        cols = np.frombuffer(key, np.int32)
        tab[np.arange(128), b * W + cols] = BF16(1.0)

    # per-graph reciprocal counts (reference divides by max(cnt, 1))
    rec_all = (1.0 / np.maximum(sizes, 1.0)).astype(np.float32)

    pay8 = np.concatenate([x, r32], axis=1).astype(F8)         # [N, C]

    in_maps = []
    perms = []
    # shared constant tiles
    wbase_b = np.broadcast_to(W_base[:, 0][None, :], (128, D)).astype(np.float32).copy()
    v1_b = np.broadcast_to(W3[0, :][None, :], (128, 32)).astype(np.float32).copy()
    m2_b = np.broadcast_to(M2.reshape(-1)[None, :], (128, R * 32)).astype(np.float32).copy()
    v0_b = np.broadcast_to(v0[None, :], (128, 32)).astype(np.float32).copy()
    w4_b = np.broadcast_to(W4[:, 0][None, :], (128, 32)).astype(np.float32).copy()
    bb_t = np.full((128, 1), float(b_base.reshape(-1)[0]), np.float32)
    b4_t = np.full((128, 1), float(b4.reshape(-1)[0]), np.float32)

    for k in range(N_CORES):
        wblk = np.zeros((WPC, T_w * 128, C), F8)
        perm = np.empty(GPC, np.int64)          # psum partition -> global graph
        for wi in range(WPC):
            gw = k * WPC + wi          # global window index
            for j in range(W):
                g = gw * W + int(order[gw, j])  # graph in slot j
                s0, e0 = int(gcut[g]), int(gcut[g + 1])
                wblk[wi, B[j] : B[j] + (e0 - s0)] = pay8[s0:e0]
                perm[wi * W + j] = g
        perms.append(perm)
        rec_c = np.ascontiguousarray(rec_all[perm][:, None])   # [128, 1]
        # [WPC, T_w, 128, C] -> issue order [T_w, WPC, 128, C] -> [n_tiles,128,C]
        tiles = wblk.reshape(WPC, T_w, 128, C).transpose(1, 0, 2, 3) \
                    .reshape(n_tiles, 128, C)
        # permute so each SBUF partition line is contiguous in DRAM
        xcc_p = np.ascontiguousarray(
            tiles.reshape(S, ST, 128, C).transpose(0, 2, 1, 3)
        ).reshape(S, 128, ST * C)
        in_maps.append(dict(
            xcc=xcc_p, tab=tab, rec_d=rec_c,
            wbase_b=wbase_b, v1_b=v1_b, m2_b=m2_b, v0_b=v0_b, w4_b=w4_b,
            bb_t=bb_t, b4_t=b4_t,
        ))

    nc = _build_program(S, ST, C, R, NBLK, tuple(int(b) for b in blk))

    kw = {}
    if _profile is not None:
        kw = dict(trace=True, **_profile)
    res = run_bass_kernel_spmd(nc, in_maps, list(range(N_CORES)), **kw)

    out = np.empty((N_GRAPHS, 1), np.float32)
    for k in range(N_CORES):
        out[perms[k]] = res.results[k]["out"].astype(np.float32)
    if _profile is not None:
        return out, res
    return out


# revision 19
# speedup vs baseline: 1.0673x; 1.0673x over previous
"""Trainium2 Bass kernel for nn_ClockAwareGNN (segment_reduce).

Model (reference, fp32):
    gp   = segment_mean(x, batch) @ W_base + b_base            # [B, 1]
    h    = relu(clock @ W1 + b1) @ W2 + b2                     # [N, 16]
    cp   = segment_mean(h, batch)                              # [B, 16]
    out  = relu([gp | cp] @ W3 + b3) @ W4 + b4                 # [B, 1]

Everything after the segment reductions is affine in per-graph quantities, so
the heavy per-node work collapses to fused segment sums:
    Sx[g]  = sum of x rows in graph g          (128 cols)
    Sr[g]  = sum of r rows in graph g          (R cols)
where r is the raw clock (R=1; exact when b1 == 0 and clock >= 0 since
relu(c*W1) == c*relu(W1) elementwise for c >= 0) or relu(clock @ W1 + b1)
(R=hid fallback). Counts are host-side bincounts (shipped as 1/cnt).

Device strategy (per core, 8-way data-parallel by graph):
  - the whole payload [x | r] ships as ONE fp8(e4m3) stream: 129 B/node.
    End-to-end quantization error is ~2.5e-3 of output scale (measured on
    the real input distribution) vs the 2e-2 gate.
  - canonical slot layout: every 32-graph window is laid out as 32
    FIXED-size node slots (slot j sized to the j-th largest graph over all
    windows; each window's graphs are placed into slots sorted by size,
    host unpermutes the gathered output).  The tile -> segment map is then
    IDENTICAL on every core, so the one-hot "assign" weights become a tiny
    static fp8 table indexed by compile-time AP offsets — no per-node
    assignment work on the device at all.
  - PE accumulates assign.T @ payload into PSUM [128 graphs, 129] fp32.
    Tiles are interleaved across the 4 windows (tile i -> window i%4) so
    consecutive matmuls land in different PSUM col-groups and overlap in
    the array (tile_position col packing).
  - tiny vector-engine epilogue computes the folded per-graph MLP.
"""

import math
import sys
import types

import numpy as np
import ml_dtypes

import concourse.bass as bass
import concourse.bacc as bacc
import concourse.tile as tile
from concourse import mybir
from concourse.bass_utils import run_bass_kernel_spmd


def _ensure_axon_hooks():
    """bass_utils' trace path does `from antenv.axon_hooks import ...`;
    some agent images lack that submodule. Install it (with the real NTFF
    hook when available) so trace=True degrades gracefully instead of
    raising ModuleNotFoundError."""
    try:
        import antenv  # noqa: F401
        import antenv.axon_hooks  # noqa: F401
        return
    except ImportError:
        pass
    try:
        import antenv
    except ImportError:
        return
    mod = types.ModuleType("antenv.axon_hooks")
    state = {"hook": None}
    mod.set_axon_ntff_profile_hook = lambda h: state.__setitem__("hook", h)
    mod.get_axon_ntff_profile_hook = lambda: state["hook"]
    sys.modules["antenv.axon_hooks"] = mod
    antenv.axon_hooks = mod
    try:
        from trn_agent_boot.trn_boot import _ntff_profile_via_ctypes
        mod.set_axon_ntff_profile_hook(
            _ntff_profile_via_ctypes("/opt/axon/libaxon_pjrt.so"))
    except Exception:
        pass
    # the trace path also uploads the NEFF dir to a bucket; in zero-egress
    # containers that raises — fall back to the local path.
    try:
        import concourse.bass_utils as _bu
        _orig_upload = _bu.upload_artifacts

        def _safe_upload(tmpdir):
            try:
                return _orig_upload(tmpdir)
            except Exception:
                return str(tmpdir)

        _bu.upload_artifacts = _safe_upload
    except Exception:
        pass


_ensure_axon_hooks()

BF16 = ml_dtypes.bfloat16
F8 = ml_dtypes.float8_e4m3

N_CORES = 8
N_GRAPHS = 1024
D = 128                 # feature dim of x
GPC = N_GRAPHS // N_CORES   # graphs per core = 128
W = 32                  # slots per window (PSUM partition alignment unit)
WPC = GPC // W          # windows per core = 4
N_WIN = N_CORES * WPC   # total windows = 32
ST = 64                 # node-tiles per DMA super-tile


def _build_program(S, ST_, C, R, NBLK, blk):
    """Build the SPMD Bass/Tile program. Shapes and the tile -> assign-block
    map are static; per-core data differences live entirely in the inputs.

    S:    number of super-tiles (each ST_ node-tiles of 128 nodes)
    C:    fp8 payload column count = 128 + R
    NBLK: number of distinct assign-weight blocks in the table
    blk:  per-in-window-tile block index (len T_w); tile i uses blk[i//WPC]
    """
    fp32 = mybir.dt.float32
    bf16 = mybir.dt.bfloat16
    f8 = mybir.dt.float8e4
    n_tiles = S * ST_

    nc = bacc.Bacc("TRN2", target_bir_lowering=False, debug=False,
                   num_devices=N_CORES)

    xcc = nc.dram_tensor("xcc", [S, 128, ST_ * C], f8, kind="ExternalInput").ap()
    tab_d = nc.dram_tensor("tab", [128, NBLK * W], bf16, kind="ExternalInput").ap()
    NC = 1 + D + 32 + R * 32 + 32 + 32 + 1 + 1   # packed epilogue constants
    cst_d = nc.dram_tensor("cst", [128, NC], fp32, kind="ExternalInput").ap()
    out_d = nc.dram_tensor("out", [128, 1], fp32, kind="ExternalOutput").ap()

    with tile.TileContext(nc) as tc:
        with (
            tc.tile_pool(name="consts", bufs=1) as cpool,
            tc.tile_pool(name="xin", bufs=8) as xpool,
            tc.tile_pool(name="epi", bufs=1) as epool,
            tc.tile_pool(name="ps", bufs=1, space="PSUM") as ppool,
        ):
            # ---- constants: assign table + ONE packed epilogue-const DMA,
            # early on the scalar ring (hidden under the payload stream) ----
            tab_t = cpool.tile([128, NBLK * W], bf16, tag="tab")
            nc.scalar.dma_start(tab_t[:], tab_d)
            cst = cpool.tile([128, NC], fp32, tag="cst")
            nc.scalar.dma_start(cst[:], cst_d)
            o = 0
            rec_t = cst[:, o : o + 1]; o += 1
            wb_t = cst[:, o : o + D]; o += D
            v1_t = cst[:, o : o + 32]; o += 32
            m2_t = cst[:, o : o + R * 32]; o += R * 32
            v0_t = cst[:, o : o + 32]; o += 32
            w4_t = cst[:, o : o + 32]; o += 32
            bbt = cst[:, o : o + 1]; o += 1
            b4t = cst[:, o : o + 1]; o += 1

            psum = ppool.tile([128, C], fp32, tag="acc")

            # init matmul: zero weights x zero rhs, start=True claims the
            # whole bank's has_written bits so all later matmuls (start=False)
            # overwrite-on-first-touch / accumulate-after, independent of
            # window interleaving.
            zw = cpool.tile([128, 128], bf16, tag="zw")
            nc.vector.memset(zw[:], 0.0)
            zr = cpool.tile([128, C], bf16, tag="zr")
            nc.vector.memset(zr[:], 0.0)
            nc.tensor.matmul(psum[:, :], zw[:], zr[:], start=True, stop=False)

            # ---- main loop ----
            for s in range(S):
                xt = xpool.tile([128, ST_ * C], f8, tag="xt")
                # alternate the two HWDGE rings so DMA fixed costs overlap
                eng = nc.sync if (s % 2 == 0) else nc.scalar
                eng.dma_start(xt[:], xcc[s])
                for t in range(ST_):
                    i = s * ST_ + t
                    w = i % WPC       # window interleave: spread col-groups
                    b = blk[i // WPC]  # static assign-weight block
                    last = i == n_tiles - 1
                    nc.tensor.matmul(
                        psum[w * W : (w + 1) * W, :],
                        tab_t[:, b * W : (b + 1) * W],
                        xt[:, t * C : (t + 1) * C],
                        start=False,
                        stop=last,
                        tile_position=(0, w * W),
                    )

            # ---- epilogue (per-graph folded MLP) ----
            sb = epool.tile([128, C], fp32, tag="sb")
            nc.vector.tensor_copy(sb[:], psum[:])

            mx = epool.tile([128, D], fp32, tag="mx")
            nc.vector.tensor_scalar_mul(mx[:], sb[:, 0:D], rec_t)
            mr = epool.tile([128, R], fp32, tag="mr")
            nc.vector.tensor_scalar_mul(mr[:], sb[:, D : D + R], rec_t)

            # gp = rowsum(mean_x * W_base) + b_base
            t1 = epool.tile([128, D], fp32, tag="t1")
            nc.vector.tensor_mul(t1[:], mx[:], wb_t)
            gp = epool.tile([128, 1], fp32, tag="gp")
            nc.vector.tensor_reduce(gp[:], t1[:], axis=mybir.AxisListType.X,
                                    op=mybir.AluOpType.add)
            nc.vector.tensor_add(gp[:], gp[:], bbt)

            # pre = gp*v1 + sum_j mr[:,j]*M2[j] + v0
            pre = epool.tile([128, 32], fp32, tag="pre")
            nc.vector.tensor_scalar_mul(pre[:], v1_t, gp[:])
            tmp = epool.tile([128, 32], fp32, tag="tmp")
            for j in range(R):
                nc.vector.tensor_scalar(
                    tmp[:], m2_t[:, j * 32 : (j + 1) * 32], mr[:, j : j + 1], None,
                    op0=mybir.AluOpType.mult,
                )
                nc.vector.tensor_add(pre[:], pre[:], tmp[:])
            nc.vector.tensor_add(pre[:], pre[:], v0_t)

            act = epool.tile([128, 32], fp32, tag="act")
            nc.scalar.activation(act[:], pre[:], mybir.ActivationFunctionType.Relu)

            # out = rowsum(act * W4) + b4
            nc.vector.tensor_mul(act[:], act[:], w4_t)
            oo = epool.tile([128, 1], fp32, tag="oo")
            nc.vector.tensor_reduce(oo[:], act[:], axis=mybir.AxisListType.X,
                                    op=mybir.AluOpType.add)
            nc.vector.tensor_add(oo[:], oo[:], b4t)

            nc.sync.dma_start(out_d, oo[:])

    nc.compile()
    return nc


def kernel(x, clock_period, batch, W_base, b_base, W1, b1, W2, b2, W3, b3, W4, b4,
           _profile=None):
    x = np.asarray(x, np.float32)
    clock = np.asarray(clock_period, np.float32).reshape(-1)
    batch = np.asarray(batch, np.int32)
    W_base = np.asarray(W_base, np.float32)
    W1 = np.asarray(W1, np.float32); b1 = np.asarray(b1, np.float32)
    W2 = np.asarray(W2, np.float32); b2 = np.asarray(b2, np.float32)
    W3 = np.asarray(W3, np.float32); b3 = np.asarray(b3, np.float32)
    W4 = np.asarray(W4, np.float32); b4 = np.asarray(b4, np.float32)
    hid = W1.shape[1]

    # r-path: exact algebraic fold when relu(c*W1 + b1) == c * relu(W1)
    fold = bool(np.all(b1 == 0.0)) and bool(clock.min() >= 0.0)
    if fold:
        R = 1
        r32 = clock[:, None]                                   # [N, 1]
        q = np.maximum(W1, 0.0) @ W2                           # [1, hid]
        M2 = q @ W3[1:, :]                                     # [1, 32]
        v0 = b2 @ W3[1:, :] + b3                               # [32]
    else:
        R = hid
        r32 = np.maximum(clock[:, None] @ W1 + b1, 0.0)        # [N, hid]
        M2 = W2 @ W3[1:, :]                                    # [hid, 32]
        v0 = b2 @ W3[1:, :] + b3

    C = D + R               # fp8 payload: [x | r]

    # ---- canonical slot layout ----
    # graph node spans + per-window size-sorted slot assignment
    gcut = np.searchsorted(batch, np.arange(N_GRAPHS + 1))
    sizes = np.diff(gcut)                                      # [1024]
    sz_w = sizes.reshape(N_WIN, W)
    order = np.argsort(-sz_w, axis=1, kind="stable")           # rank -> local graph
    sorted_sz = -np.sort(-sz_w, axis=1)                        # [N_WIN, W] desc
    L = sorted_sz.max(axis=0).astype(np.int64)                 # slot sizes [W]
    B = np.zeros(W + 1, np.int64)
    B[1:] = np.cumsum(L)                                       # slot starts
    T_w = int(math.ceil(B[W] / 128.0))
    while (WPC * T_w) % ST:
        T_w += 1
    n_tiles = WPC * T_w
    S = n_tiles // ST

    # static per-tile assign pattern: node offset o -> slot searchsorted(B,o)
    offs = np.arange(T_w * 128, dtype=np.int64)
    col_of = np.clip(np.searchsorted(B, offs, side="right") - 1, 0, W - 1)
    col_of = col_of.reshape(T_w, 128).astype(np.int32)
    pats = {}
    blk = np.empty(T_w, np.int32)
    for t in range(T_w):
        key = col_of[t].tobytes()
        if key not in pats:
            pats[key] = len(pats)
        blk[t] = pats[key]
    NBLK = len(pats)
    tab = np.zeros((128, NBLK * W), BF16)
    for key, b in pats.items():
        cols = np.frombuffer(key, np.int32)
        tab[np.arange(128), b * W + cols] = BF16(1.0)

    # per-graph reciprocal counts (reference divides by max(cnt, 1))
    rec_all = (1.0 / np.maximum(sizes, 1.0)).astype(np.float32)

    pay8 = np.concatenate([x, r32], axis=1).astype(F8)         # [N, C]

    in_maps = []
    perms = []
    # shared packed epilogue constants (rec is per-core, filled below)
    row = np.concatenate([
        np.zeros(1, np.float32),                 # rec placeholder
        W_base[:, 0].astype(np.float32),
        W3[0, :].astype(np.float32),
        M2.reshape(-1).astype(np.float32),
        v0.astype(np.float32),
        W4[:, 0].astype(np.float32),
        np.array([float(b_base.reshape(-1)[0])], np.float32),
        np.array([float(b4.reshape(-1)[0])], np.float32),
    ])
    cst_base = np.broadcast_to(row[None, :], (128, row.shape[0])).copy()

    for k in range(N_CORES):
        wblk = np.zeros((WPC, T_w * 128, C), F8)
        perm = np.empty(GPC, np.int64)          # psum partition -> global graph
        for wi in range(WPC):
            gw = k * WPC + wi          # global window index
            for j in range(W):
                g = gw * W + int(order[gw, j])  # graph in slot j
                s0, e0 = int(gcut[g]), int(gcut[g + 1])
                wblk[wi, B[j] : B[j] + (e0 - s0)] = pay8[s0:e0]
                perm[wi * W + j] = g
        perms.append(perm)
        cst_c = cst_base.copy()
        cst_c[:, 0] = rec_all[perm]
        # [WPC, T_w, 128, C] -> issue order [T_w, WPC, 128, C] -> [n_tiles,128,C]
        tiles = wblk.reshape(WPC, T_w, 128, C).transpose(1, 0, 2, 3) \
                    .reshape(n_tiles, 128, C)
        # permute so each SBUF partition line is contiguous in DRAM
        xcc_p = np.ascontiguousarray(
            tiles.reshape(S, ST, 128, C).transpose(0, 2, 1, 3)
        ).reshape(S, 128, ST * C)
        in_maps.append(dict(xcc=xcc_p, tab=tab, cst=cst_c))

    nc = _build_program(S, ST, C, R, NBLK, tuple(int(b) for b in blk))

    kw = {}
    if _profile is not None:
        kw = dict(trace=True, **_profile)
    res = run_bass_kernel_spmd(nc, in_maps, list(range(N_CORES)), **kw)

    out = np.empty((N_GRAPHS, 1), np.float32)
    for k in range(N_CORES):
        out[perms[k]] = res.results[k]["out"].astype(np.float32)
    if _profile is not None:
        return out, res
    return out
